# revision 4
# baseline (speedup 1.0000x reference)
"""BnBinActiveConv2d Trainium2 kernel, v2.

Pipeline (per reference):
  BN (batch stats, train mode) -> BinActive (sign + K = box(mean_c |xn|))
  -> BinConv (sign weights) -> relu(y * K * alpha)

Changes vs v1:
  - Weights pre-signed/transposed to fp8 and alpha pre-reduced on host
    (kills the 4.7MB wt/wo loads and the prologue sign/reduce work).
  - y stored bf16 (halves output DMA), upcast on host.
  - K path in bf16; broadcast DMA is 1/4 of v1's f32 broadcast.
  - PSUM evacuation is ONE DVE op per super-group:
    y = max(psum,0) * (K*alpha), with K*alpha folded on the otherwise
    idle Pool engine per 18-row super-group.
  - ad (|xn|) stored fp8 interleaved [P,2,HW]: the A channel-sum uses
    DoubleRow matmuls; psA chunks drained by DVE paced between evacs.
  - Single allreduce of both chunks' (mean, E[x^2]); derivation
    vectorized over the two channel chunks, no Newton steps.
  - The last x tile is loaded in 7 chunk-DMAs so its bn_stats tail is
    ~1us instead of ~4.
  - Per-image Act order (abs,abs,sign,sign) so image 0's conv starts
    ~11us after the stats land, and each conv(n) overlaps acts(n+1).

Sharding: data-parallel over batch, only BN partial stats (2KB) are
all-reduced.
"""

import numpy as np
from contextlib import ExitStack

import concourse.bass as bass
import concourse.bacc as bacc
import concourse.tile as tile
from concourse import mybir
from concourse.alu_op_type import AluOpType

AFT = mybir.ActivationFunctionType
FP32 = mybir.dt.float32
BF16 = mybir.dt.bfloat16
FP8 = mybir.dt.float8e4
DR = mybir.MatmulPerfMode.DoubleRow

BN_EPS = 1e-4
P = 128


def build(B_loc=4, C=256, H=56, KS=3, n_cores=8, fp8=True, loop_k=1,
          local=False, taporder=False, convmode=1,
          pool_ka=False, pool_box=False):
    """convmode: 0 = group-outer/tap-inner (default), 1 = tap-outer within
    each super-group (2 matmuls per weight load), 2 = co-outer/tap-outer
    (6 matmuls per weight load; all 3 super-group psum tiles live)."""
    if taporder:
        convmode = 1
    assert fp8
    W_ = H
    HO, WO = H - KS + 1, W_ - KS + 1
    CC = C // P
    HW, HOWO = H * W_, HO * WO
    HWP = HW + 64                     # padded stride for xs8 (junk cols)
    NJ, JN = 448, HW // 448           # A-sum / bn_stats chunking
    GPS = 1 if convmode == 3 else 2   # 9-row groups per psum tile
    NSG = HO // (GPS * 9)             # super-groups per image
    RSG = HO // NSG                   # rows per super-group
    EV = RSG * WO                     # evac elems per super-group
    assert NSG * RSG == HO and RSG == GPS * 9

    nc = bacc.Bacc("TRN2", target_bir_lowering=False, debug=False,
                   enable_asserts=False, num_devices=n_cores)

    x_d = nc.dram_tensor("x", [B_loc, C, H, W_], FP32, kind="ExternalInput").ap()
    wb_d = nc.dram_tensor("wb", [P, CC, KS * KS * C], FP8, kind="ExternalInput").ap()
    al_d = nc.dram_tensor("alpha", [C], FP32, kind="ExternalInput").ap()
    g_d = nc.dram_tensor("gamma", [C], FP32, kind="ExternalInput").ap()
    b_d = nc.dram_tensor("beta", [C], FP32, kind="ExternalInput").ap()
    y_d = nc.dram_tensor("y", [B_loc, C, HO, WO], BF16, kind="ExternalOutput").ap()

    with tile.TileContext(nc) as tc:
        with ExitStack() as ctx:
            consts = ctx.enter_context(tc.tile_pool(name="consts", bufs=1))
            statsp = ctx.enter_context(tc.tile_pool(name="stats", bufs=1))
            dram = ctx.enter_context(tc.tile_pool(name="dram", bufs=1, space="DRAM"))

            # ---- constants / weights ----
            wb8 = consts.tile([P, CC, KS * KS * C], FP8)
            nc.sync.dma_start(out=wb8, in_=wb_d)
            wb8v = wb8.rearrange("p c (t o) -> p c t o", o=C)
            alpha = consts.tile([P, CC], FP32)
            nc.sync.dma_start(out=alpha, in_=al_d.rearrange("(cc p) -> p cc", p=P))
            gam = consts.tile([P, CC], FP32)
            nc.sync.dma_start(out=gam, in_=g_d.rearrange("(cc p) -> p cc", p=P))
            bet = consts.tile([P, CC], FP32)
            nc.sync.dma_start(out=bet, in_=b_d.rearrange("(cc p) -> p cc", p=P))
            # ones for the A-sum DR matmul; free dim padded to 16 so the
            # k-tile dim stride satisfies the DoubleRow step%16 rule.
            ones8 = consts.tile([P, CC, 16], FP8)
            nc.vector.memset(ones8, 1.0)
            eps_sb = consts.tile([P, CC], FP32)
            nc.vector.memset(eps_sb, BN_EPS)

            s_sb = consts.tile([P, CC], FP32)
            neg_t = consts.tile([P, CC], FP32)
            nst = consts.tile([P, CC], FP32)

            xpool = ctx.enter_context(tc.tile_pool(name="x", bufs=1))
            xsp = ctx.enter_context(tc.tile_pool(name="xs", bufs=1))
            adp = ctx.enter_context(tc.tile_pool(name="ad", bufs=1))
            arp = ctx.enter_context(tc.tile_pool(name="ar", bufs=2))
            boxp = ctx.enter_context(tc.tile_pool(name="box", bufs=2))
            kbp = ctx.enter_context(tc.tile_pool(name="kb", bufs=3))
            kap = ctx.enter_context(tc.tile_pool(name="ka", bufs=3))
            yp = ctx.enter_context(tc.tile_pool(name="y", bufs=4))
            psA = ctx.enter_context(tc.tile_pool(name="psA", bufs=2, space="PSUM"))
            psC = ctx.enter_context(tc.tile_pool(
                name="psC", bufs=(6 if convmode == 3 else 3), space="PSUM"))

            kflat_d = dram.tile([B_loc, HOWO], BF16)
            cc_in = dram.tile([P, 2 * CC], FP32)
            cc_out = dram.tile([P, 2 * CC], FP32)

            for rep in range(loop_k):
                x_sb, ad, xs8, a_row = {}, {}, {}, {}
                for n in range(B_loc):
                    ad[n] = adp.tile([P, CC, HW], FP8, tag=f"ad{n}",
                                     name=f"ad{n}r{rep}")
                    xs8[n] = xsp.tile([P, CC, HWP], FP8, tag=f"xs{n}",
                                      name=f"xs{n}r{rep}")
                    nc.gpsimd.memset(xs8[n][:, :, HW:HWP], 0.0)

                # ---- phase 1: load x, bn_stats (no derive in between) ----
                stats = {}
                for cc in range(CC):
                    stats[cc] = statsp.tile([P, B_loc * JN, 6], FP32,
                                            tag=f"st{cc}", name=f"st{cc}r{rep}")
                order = [(n, cc) for cc in range(CC) for n in range(B_loc)]
                for (n, cc) in order:
                    xt = xpool.tile([P, HW], FP32, tag=f"x{n}{cc}",
                                    name=f"xt{n}{cc}r{rep}")
                    src = x_d[n, cc * P:(cc + 1) * P].rearrange("c h w -> c (h w)")
                    xr = xt.rearrange("p (j v) -> p j v", v=NJ)
                    last_tile = (n, cc) == order[-1]
                    if last_tile:
                        # chunked load so the stats tail is ~1 chunk long
                        for j in range(JN):
                            nc.sync.dma_start(out=xr[:, j, :],
                                              in_=src[:, j * NJ:(j + 1) * NJ])
                            nc.vector.bn_stats(out=stats[cc][:, n * JN + j, :],
                                               in_=xr[:, j, :])
                    else:
                        nc.sync.dma_start(out=xt, in_=src)
                        for j in range(JN):
                            nc.vector.bn_stats(out=stats[cc][:, n * JN + j, :],
                                               in_=xr[:, j, :])
                    x_sb[n, cc] = xt

                # ---- phase 2: one allreduce, vectorized derivation ----
                pk = statsp.tile([P, 2 * CC], FP32, tag="pk", name=f"pkr{rep}")
                pkv = pk.rearrange("p (c k) -> p c k", k=2)
                for cc in range(CC):
                    mv = statsp.tile([P, 2], FP32, tag=f"mv{cc}",
                                     name=f"mv{cc}r{rep}")
                    nc.vector.bn_aggr(out=mv, in_=stats[cc])
                    nc.vector.tensor_copy(out=pkv[:, cc, 0:1], in_=mv[:, 0:1])
                    # m2 = var + mean^2
                    nc.vector.scalar_tensor_tensor(
                        out=pkv[:, cc, 1:2], in0=mv[:, 0:1], scalar=mv[:, 0:1],
                        in1=mv[:, 1:2], op0=AluOpType.mult, op1=AluOpType.add)
                nc.scalar.dma_start(out=cc_in, in_=pk)
                if local:
                    nc.scalar.dma_start(out=cc_out, in_=cc_in)
                else:
                    nc.gpsimd.collective_compute(
                        "AllReduce", AluOpType.add,
                        replica_groups=[list(range(n_cores))],
                        ins=[cc_in.opt()], outs=[cc_out.opt()])
                sums = statsp.tile([P, 2 * CC], FP32, tag="su", name=f"sur{rep}")
                nc.scalar.dma_start(out=sums, in_=cc_out)
                nc.vector.tensor_scalar_mul(sums, sums, 1.0 / n_cores)
                sv = sums.rearrange("p (c k) -> p c k", k=2)
                mean = sv[:, :, 0]                      # [P, CC] strided
                var = statsp.tile([P, CC], FP32, tag="va", name=f"var{rep}")
                msq = statsp.tile([P, CC], FP32, tag="ms", name=f"msr{rep}")
                # var = E[x^2] - mean^2
                nc.vector.tensor_tensor(out=msq, in0=mean, in1=mean,
                                        op=AluOpType.mult)
                nc.vector.tensor_tensor(out=var, in0=sv[:, :, 1], in1=msq,
                                        op=AluOpType.subtract)
                r = statsp.tile([P, CC], FP32, tag="r", name=f"rr{rep}")
                nc.vector.tensor_scalar_add(r, var, BN_EPS)
                nc.scalar.activation(out=r, in_=r, func=AFT.Sqrt)
                nc.vector.reciprocal(out=r, in_=r)
                nc.vector.tensor_tensor(out=s_sb, in0=r, in1=gam,
                                        op=AluOpType.mult)
                inv_s = statsp.tile([P, CC], FP32, tag="is", name=f"isr{rep}")
                nc.vector.reciprocal(out=inv_s, in_=s_sb)
                nc.vector.tensor_tensor(out=inv_s, in0=bet, in1=inv_s,
                                        op=AluOpType.mult)
                nc.vector.tensor_tensor(out=neg_t, in0=inv_s, in1=mean,
                                        op=AluOpType.subtract)
                nc.vector.tensor_tensor(out=nst, in0=s_sb, in1=neg_t,
                                        op=AluOpType.mult)

                # per-image binarize: image n fully ready after 4 Act ops
                for n in range(B_loc):
                    for cc in range(CC):
                        nc.scalar.activation(out=ad[n][:, cc, :],
                                             in_=x_sb[n, cc], func=AFT.Abs,
                                             bias=nst[:, cc:cc + 1],
                                             scale=s_sb[:, cc:cc + 1])
                    for cc in range(CC):
                        nc.scalar.activation(out=xs8[n][:, cc, 0:HW],
                                             in_=x_sb[n, cc], func=AFT.Sign,
                                             bias=neg_t[:, cc:cc + 1])

                # ---- phase 3 helpers ----
                def emit_A_chunk(n, j):
                    """One A channel-sum chunk -> psA; drained by DVE."""
                    if j == 0:
                        a_row[n] = arp.tile([1, HW], BF16, tag="arow",
                                            name=f"arow{n}r{rep}")
                    pa = psA.tile([1, NJ], FP32, tag="psA")
                    nc.tensor.matmul(pa, lhsT=ones8[:, :, 0:1],
                                     rhs=ad[n][:, :, j * NJ:(j + 1) * NJ],
                                     start=True, stop=True, perf_mode=DR)
                    nc.vector.tensor_copy(out=a_row[n][:, j * NJ:(j + 1) * NJ],
                                          in_=pa)

                def emit_kpath(n):
                    """a_row -> shifted tiles -> box (Pool) -> kflat -> kb/ka."""
                    arv = a_row[n].rearrange("p (h w) -> p h w", w=W_)
                    a_sh = [boxp.tile([HO, W_], BF16, tag=f"ash{k}",
                                      name=f"ash{n}{k}r{rep}")
                            for k in range(KS)]
                    for k in range(KS):
                        nc.sync.dma_start(out=a_sh[k], in_=arv[:, k:k + HO, :])
                    box_eng = nc.gpsimd if pool_box else nc.vector
                    t1 = boxp.tile([HO, W_], FP32, tag="t1", name=f"t1{n}r{rep}")
                    box_eng.tensor_tensor(out=t1, in0=a_sh[0], in1=a_sh[1],
                                          op=AluOpType.add)
                    box_eng.tensor_tensor(out=t1, in0=t1, in1=a_sh[2],
                                          op=AluOpType.add)
                    ks1 = boxp.tile([HO, WO], FP32, tag="ks1",
                                    name=f"ks1{n}r{rep}")
                    box_eng.tensor_tensor(out=ks1, in0=t1[:, 0:WO],
                                          in1=t1[:, 1:WO + 1],
                                          op=AluOpType.add)
                    k_im = boxp.tile([HO, WO], BF16, tag="kim",
                                     name=f"kim{n}r{rep}")
                    box_eng.tensor_tensor(out=k_im, in0=ks1,
                                          in1=t1[:, 2:WO + 2],
                                          op=AluOpType.add)
                    nc.sync.dma_start(
                        out=kflat_d[n, :].rearrange("(h w) -> h w", w=WO),
                        in_=k_im)
                    kas = {}
                    for sg in range(NSG):
                        kb = kbp.tile([P, EV], BF16, tag="kb",
                                      name=f"kb{n}{sg}r{rep}")
                        ksrc = kflat_d[n, sg * EV:(sg + 1) * EV]
                        nc.gpsimd.dma_start(
                            out=kb,
                            in_=bass.AP(tensor=ksrc.tensor, offset=ksrc.offset,
                                        ap=[[0, P]] + list(ksrc.ap)))
                        for co in range(CC):
                            ka = kap.tile([P, EV], BF16, tag=f"ka{co}",
                                          name=f"ka{co}{n}{sg}r{rep}")
                            ka_eng = nc.vector if (n == 0 or not pool_ka) \
                                else nc.gpsimd
                            ka_eng.tensor_scalar(
                                out=ka, in0=kb,
                                scalar1=alpha[:, co:co + 1], scalar2=None,
                                op0=AluOpType.mult)
                            kas[sg, co] = ka
                    return kas

                def emit_evac(n, kas, co, sg, pc):
                    y_t = yp.tile([P, EV], BF16, tag="y")
                    GEV = EV // GPS
                    for gb in range(GPS):
                        pcg = pc[:, gb, 0:9 * W_].rearrange(
                            "p (b w) -> p b w", w=W_)
                        nc.vector.scalar_tensor_tensor(
                            out=y_t[:, gb * GEV:(gb + 1) * GEV].rearrange(
                                "p (b w) -> p b w", w=WO),
                            in0=pcg[:, :, 0:WO], scalar=0.0,
                            in1=kas[sg, co][:, gb * GEV:(gb + 1) * GEV]
                                .rearrange("p (b w) -> p b w", w=WO),
                            op0=AluOpType.max, op1=AluOpType.mult)
                    nc.sync.dma_start(
                        out=y_d[n, co * P:(co + 1) * P,
                                sg * RSG:(sg + 1) * RSG, :].rearrange(
                                    "c h w -> c (h w)"),
                        in_=y_t)

                def emit_conv_sg(n, kas, co, sg):
                    # each 9-row group gets a full 512-f32 PSUM bank
                    pc = psC.tile([P, GPS, 512], FP32, tag="psC")
                    if convmode == 1:
                        # tap-outer: consecutive matmuls share the same lhsT
                        for t in range(KS * KS):
                            kh, kw = divmod(t, KS)
                            for gb in range(GPS):
                                g = sg * GPS + gb
                                off = g * 9 * W_ + kh * W_ + kw
                                nc.tensor.matmul(
                                    pc[:, gb, 0:9 * W_],
                                    lhsT=wb8v[:, :, t, co * P:(co + 1) * P],
                                    rhs=xs8[n][:, :, off:off + 9 * W_],
                                    start=(t == 0), stop=(t == KS * KS - 1),
                                    perf_mode=DR)
                    else:
                        for gb in range(GPS):
                            g = sg * GPS + gb
                            first = True
                            for kh in range(KS):
                                for kw in range(KS):
                                    last = (kh == KS - 1 and kw == KS - 1)
                                    off = g * 9 * W_ + kh * W_ + kw
                                    nc.tensor.matmul(
                                        pc[:, gb, 0:9 * W_],
                                        lhsT=wb8v[:, :, kh * KS + kw,
                                                  co * P:(co + 1) * P],
                                        rhs=xs8[n][:, :, off:off + 9 * W_],
                                        start=first, stop=last, perf_mode=DR)
                                    first = False
                    emit_evac(n, kas, co, sg, pc)

                def emit_conv_co(n, kas, co, paceA):
                    """co-outer: all 3 super-groups accumulate tap-by-tap;
                    each weight load streams 6 matmuls. paceA: list of
                    (tap_idx -> A chunks of next image to interleave)."""
                    pcs = [psC.tile([P, GPS, 512], FP32, tag="psC",
                                    name=f"pc{n}c{co}s{s}r{rep}")
                           for s in range(NSG)]
                    for t in range(KS * KS):
                        kh, kw = divmod(t, KS)
                        for sg in range(NSG):
                            for gb in range(GPS):
                                g = sg * GPS + gb
                                off = g * 9 * W_ + kh * W_ + kw
                                nc.tensor.matmul(
                                    pcs[sg][:, gb, 0:9 * W_],
                                    lhsT=wb8v[:, :, t, co * P:(co + 1) * P],
                                    rhs=xs8[n][:, :, off:off + 9 * W_],
                                    start=(t == 0), stop=(t == KS * KS - 1),
                                    perf_mode=DR)
                        for _ in range(paceA.pop(0) if paceA else 0):
                            yield None
                    for sg in range(NSG):
                        emit_evac(n, kas, co, sg, pcs[sg])

                # ---- phase 3: conv(n) with A(n+1) interleaved on PE ----
                for j in range(JN):
                    emit_A_chunk(0, j)
                kas = emit_kpath(0)
                if convmode == 2:
                    for n in range(B_loc):
                        nxt = n + 1
                        ai = [0]

                        def a_sink():
                            if nxt < B_loc and ai[0] < JN:
                                emit_A_chunk(nxt, ai[0])
                                ai[0] += 1

                        for co in range(CC):
                            # interleave one A chunk of next img per tap slot
                            pace = [1, 1, 1, 1, 0, 0, 0, 0, 0] if co == 0 \
                                else [1, 1, 1, 0, 0, 0, 0, 0, 0]
                            gen = emit_conv_co(n, kas, co, pace)
                            for _ in gen:
                                a_sink()
                        if nxt < B_loc:
                            while ai[0] < JN:
                                emit_A_chunk(nxt, ai[0])
                                ai[0] += 1
                            kas = emit_kpath(nxt)
                else:
                    seq = [(co, sg) for co in range(CC) for sg in range(NSG)]
                    if len(seq) == 6:
                        pace = [2, 2, 1, 1, 1, 0]  # A-chunks of next img
                    else:
                        pace = [1] * JN + [0] * (len(seq) - JN)
                    for n in range(B_loc):
                        nxt = n + 1
                        ai = 0
                        for i, (co, sg) in enumerate(seq):
                            emit_conv_sg(n, kas, co, sg)
                            if nxt < B_loc:
                                for _ in range(pace[i]):
                                    emit_A_chunk(nxt, ai)
                                    ai += 1
                        if nxt < B_loc:
                            kas = emit_kpath(nxt)

    nc.compile()
    return nc


_CACHE = {}


def _get_compiled():
    if "nc" not in _CACHE:
        _CACHE["nc"] = build()
    return _CACHE["nc"]


def make_in_maps(x, gamma, beta, W, n_cores=8):
    x = np.ascontiguousarray(np.asarray(x, dtype=np.float32))
    gamma = np.ascontiguousarray(np.asarray(gamma, dtype=np.float32))
    beta = np.ascontiguousarray(np.asarray(beta, dtype=np.float32))
    W = np.asarray(W, dtype=np.float32)
    C = W.shape[0]
    KS = W.shape[2]
    CC = C // P
    # sign(W) in the DoubleRow lhsT layout [ci_lo, ci_chunk, (kh kw co)]
    S = np.sign(W).transpose(1, 2, 3, 0)          # [ci, kh, kw, co]
    S = S.reshape(CC, P, KS, KS, C)               # [cc, p, kh, kw, co]
    S = S.transpose(1, 0, 2, 3, 4)                # [p, cc, kh, kw, co]
    wb = np.ascontiguousarray(
        S.reshape(P, CC, KS * KS * C).astype(mybir.dt.np(FP8)))
    # alpha/(C*KS^2) with the raw box-sum normalization folded in
    ckk = C * KS * KS
    alpha = np.ascontiguousarray(
        (np.abs(W).sum(axis=(1, 2, 3)) / (ckk * ckk)).astype(np.float32))
    B_loc = x.shape[0] // n_cores
    return [
        {"x": np.ascontiguousarray(x[c * B_loc:(c + 1) * B_loc]),
         "wb": wb, "alpha": alpha, "gamma": gamma, "beta": beta}
        for c in range(n_cores)
    ]


def run(x, gamma, beta, W, trace=False):
    from concourse import bass_utils
    nc = _get_compiled()
    in_maps = make_in_maps(x, gamma, beta, W)
    res = bass_utils.run_bass_kernel_spmd(nc, in_maps, core_ids=list(range(8)),
                                          trace=trace)
    out = np.concatenate(
        [r["y"].astype(np.float32) for r in res.results], axis=0)
    return out, res


def kernel(x, gamma, beta, W):
    out, _ = run(x, gamma, beta, W)
    return out
